# revision 1
# baseline (speedup 1.0000x reference)
"""Trainium2 Bass kernel for the Dynamic MultiTeacher distillation loss.

Strategy (data-parallel over 8 NeuronCores, 1024 rows each):

Device (per core, per 128-row tile), all f32:
  - PE: ps1 = x1+x2+x3+x4 (= 4*mimic) and ps2 = ps1 - 4*s, both accumulated
    in PSUM via identity matmuls (the /4 is folded into downstream scales).
  - ScalarE: e_t = exp(x_t / 20) with fused accumulate S_t = sum(e_t)
    (max-subtraction is skipped: |x|/20 <= ~0.3 so exp is safe);
    likewise Sum(exp(s)) and Sum(exp(s/20)) for CE / KD log-partitions.
  - GPSIMD (Pool): diff_t = x_t - s for the four real teachers.
  - VectorE: top-8 per row per teacher (row max m1, second max m2), and
    one fused dot per teacher D_t = sum(e_t * diff_t) via
    tensor_tensor_reduce (the mimic uses ps2 with scale 1/4... see below).

Host (tiny O(B) work in f64):
  - gathers x_t[i, target_i] (exact), the three global scalar reductions
    (min gathered -> shift, max logit -> max_preds, final mean),
  - margins = relu(gathered - m2)  [provably equal to the reference's
    "top1==gathered ? top1-top2 : 0" including ties],
  - KD_t = T*D_t/S_t + T^2*(lse_s - lse_t), CE = lse1 - s_gathered,
  - threshold softmax, w1/w2 blend, mean.
"""

import numpy as np

N_CORES = 8
B_FULL = 8192
C_DIM = 1000
B_LOC = B_FULL // N_CORES          # 1024 rows per core
P = 128                            # partitions
N_TILES = B_LOC // P               # 8 row-tiles per core

T_KD = 20.0
T_THR = 6.0
EPS = 1e-05

# device output column layout: [P, 53]
#   cols 8t..8t+7 : top8 of teacher t (t=0..3 real, t=4 is 4*mimic)
#   col 40+t      : S_t  = sum exp(x_t/20)         (t=4 from ps1, scale 1/80)
#   col 45+t      : D_t  = sum e_t * (x_t - s)     (t<4)
#   col 49        : sum e_m * ps1   (host: D_m = col49/4 - col52)
#   col 50        : sum exp(s)
#   col 51        : sum exp(s/20)
#   col 52        : sum e_m * s
OUT_COLS = 53

_CACHE = {}


def _build_nc():
    import concourse.bacc as bacc
    import concourse.mybir as mybir
    from concourse import tile

    nc = bacc.Bacc(
        "TRN2",
        target_bir_lowering=False,
        debug=False,
        num_devices=N_CORES,
    )
    f32 = mybir.dt.float32
    Alu = mybir.AluOpType
    Act = mybir.ActivationFunctionType

    xs = [
        nc.dram_tensor(f"x{t}", [B_LOC, C_DIM], f32, kind="ExternalInput").ap()
        for t in range(4)
    ]
    s_dram = nc.dram_tensor("s", [B_LOC, C_DIM], f32, kind="ExternalInput").ap()
    ident = nc.dram_tensor("ident", [P, P], f32, kind="ExternalInput").ap()
    negid = nc.dram_tensor("negid", [P, P], f32, kind="ExternalInput").ap()
    res = nc.dram_tensor("res", [B_LOC, OUT_COLS], f32, kind="ExternalOutput").ap()

    with tile.TileContext(nc) as tc:
        with (
            tc.tile_pool(name="const", bufs=1) as cpool,
            tc.tile_pool(name="io", bufs=5) as xpool,
            tc.tile_pool(name="exps", bufs=8) as epool,
            tc.tile_pool(name="diffs", bufs=3) as dfpool,
            tc.tile_pool(name="sink", bufs=2) as spool,
            tc.tile_pool(name="outs", bufs=3) as opool,
            tc.tile_pool(name="dump", bufs=8) as dpool,
            tc.tile_pool(name="ps", bufs=2, space="PSUM") as pspool,
            tc.tile_pool(name="psd", bufs=1, space="PSUM") as psdpool,
        ):
            id_tile = cpool.tile([P, P], f32, tag="id")
            nc.sync.dma_start(out=id_tile[:], in_=ident)
            nid_tile = cpool.tile([P, P], f32, tag="nid")
            nc.sync.dma_start(out=nid_tile[:], in_=negid)

            for i in range(N_TILES):
                r0 = i * P
                st = xpool.tile([P, C_DIM], f32, tag="s")
                nc.sync.dma_start(out=st[:], in_=s_dram[r0 : r0 + P, :])
                xt_tiles = []
                for t in range(4):
                    xt = xpool.tile([P, C_DIM], f32, tag=f"x{t}")
                    dma_eng = nc.sync if t < 2 else nc.scalar
                    dma_eng.dma_start(out=xt[:], in_=xs[t][r0 : r0 + P, :])
                    xt_tiles.append(xt)

                out_t = opool.tile([P, OUT_COLS], f32)

                # lse sums for the student
                sink1 = spool.tile([P, C_DIM], f32, tag="sink")
                nc.scalar.activation(
                    sink1[:], st[:], Act.Exp, scale=1.0,
                    accum_out=out_t[:, 50:51],
                )
                sink2 = spool.tile([P, C_DIM], f32, tag="sink")
                nc.scalar.activation(
                    sink2[:], st[:], Act.Exp, scale=1.0 / T_KD,
                    accum_out=out_t[:, 51:52],
                )

                # ---- Pool: diffs for teachers 0,1 (overlap DVE's max8 phase) ----
                diff_aps = [None] * 4
                for t in (0, 1):
                    df = dfpool.tile([P, C_DIM], f32, tag=f"df{t}")
                    nc.gpsimd.tensor_tensor(
                        out=df[:], in0=xt_tiles[t][:], in1=st[:], op=Alu.subtract
                    )
                    diff_aps[t] = df[:]

                # ---- PE: ps1 = x1+x2+x3+x4; PSUM diffs for teachers 2,3 ----
                ps1 = pspool.tile([P, C_DIM], f32, tag="ps1")
                psd2 = psdpool.tile([P, C_DIM], f32, tag="psd2")
                psd3 = psdpool.tile([P, C_DIM], f32, tag="psd3")
                psd = {2: psd2, 3: psd3}
                for c0, c1 in ((0, 512), (512, C_DIM)):
                    for t in (2, 3):
                        nc.tensor.matmul(
                            psd[t][:, c0:c1], id_tile[:], xt_tiles[t][:, c0:c1],
                            start=True, stop=False,
                        )
                        nc.tensor.matmul(
                            psd[t][:, c0:c1], nid_tile[:], st[:, c0:c1],
                            start=False, stop=True,
                        )
                for c0, c1 in ((0, 512), (512, C_DIM)):
                    for t in range(4):
                        nc.tensor.matmul(
                            ps1[:, c0:c1], id_tile[:], xt_tiles[t][:, c0:c1],
                            start=(t == 0), stop=(t == 3),
                        )
                diff_aps[2] = psd[2][:]
                diff_aps[3] = psd[3][:]

                streams = [xt[:] for xt in xt_tiles] + [ps1[:]]
                escale = [1.0 / T_KD] * 4 + [1.0 / (4.0 * T_KD)]

                for t in range(5):
                    # top-8 (m1, m2 live in cols 8t, 8t+1)
                    nc.vector.max(out=out_t[:, 8 * t : 8 * t + 8], in_=streams[t])
                    # e_t = exp(src * escale), S_t fused
                    et = epool.tile([P, C_DIM], f32, tag="e")
                    nc.scalar.activation(
                        et[:], streams[t], Act.Exp,
                        scale=escale[t],
                        accum_out=out_t[:, 40 + t : 41 + t],
                    )
                    if t < 4:
                        # D_t = sum(e * diff)   (VectorE, single-pass fused)
                        dA = dpool.tile([P, 1], f32, tag="dA")
                        nc.vector.scalar_tensor_tensor(
                            out=dA.broadcast_to([P, C_DIM]),
                            in0=et[:],
                            scalar=0.0,
                            in1=diff_aps[t],
                            op0=Alu.bypass,
                            op1=Alu.mult,
                            accum_out=out_t[:, 45 + t : 46 + t],
                        )
                    else:
                        # mimic: two dots; host combines D_m = col49/4 - col52
                        dA = dpool.tile([P, 1], f32, tag="dA")
                        nc.vector.scalar_tensor_tensor(
                            out=dA.broadcast_to([P, C_DIM]),
                            in0=et[:],
                            scalar=0.0,
                            in1=ps1[:],
                            op0=Alu.bypass,
                            op1=Alu.mult,
                            accum_out=out_t[:, 49:50],
                        )
                        dB = dpool.tile([P, 1], f32, tag="dB")
                        nc.vector.scalar_tensor_tensor(
                            out=dB.broadcast_to([P, C_DIM]),
                            in0=et[:],
                            scalar=0.0,
                            in1=st[:],
                            op0=Alu.bypass,
                            op1=Alu.mult,
                            accum_out=out_t[:, 52:53],
                        )

                nc.sync.dma_start(out=res[r0 : r0 + P, :], in_=out_t[:])

    nc.finalize()
    return nc


def _get_nc():
    if "nc" not in _CACHE:
        _CACHE["nc"] = _build_nc()
    return _CACHE["nc"]


def _run_device(in_maps, trace=False):
    from concourse.bass_utils import run_bass_kernel_spmd

    nc = _get_nc()
    return run_bass_kernel_spmd(
        nc, in_maps, core_ids=list(range(N_CORES)), trace=trace
    )


def _host_combine(res_cores, g, g_s):
    """res_cores: [N_CORES][B_LOC, OUT_COLS] f32; g: [B,4] gathered teacher
    logits (f64); g_s: [B] gathered student logits (f64)."""
    r = np.concatenate(res_cores, axis=0).astype(np.float64)  # [B, 52]

    g_m = g.mean(axis=1)                                     # mimic gathered
    gathered = np.concatenate([g, g_m[:, None]], axis=1)     # [B,5]

    m1 = r[:, [0, 8, 16, 24, 32]].copy()
    m2 = r[:, [1, 9, 17, 25, 33]].copy()
    m1[:, 4] *= 0.25
    m2[:, 4] *= 0.25
    S = r[:, 40:45]
    D = r[:, 45:50].copy()
    D[:, 4] = r[:, 49] * 0.25 - r[:, 52]
    S1 = r[:, 50]
    S20 = r[:, 51]

    Cmin = g.min()
    shift = (-Cmin + EPS) if Cmin < 0 else 0.0

    margins = np.maximum(gathered - m2, 0.0)
    z = margins / T_THR
    z = z - z.max(axis=1, keepdims=True)
    ez = np.exp(z)
    thr = ez / ez.sum(axis=1, keepdims=True)

    max_preds = m1[:, :4].max() + shift

    lse_t = np.log(S)
    KD = T_KD * D / S + (T_KD * T_KD) * (np.log(S20)[:, None] - lse_t)
    CE = np.log(S1) - g_s

    w2 = (gathered + shift) / max_preds
    losses = (1.0 - w2) * CE[:, None] + w2 * KD
    return np.asarray((thr * losses).sum(axis=1).mean(), dtype=np.float32)


def kernel(outputs1, outputs2, outputs3, outputs4, out_s, targets,
           _trace=False, _return_results=False):
    xs = [np.ascontiguousarray(np.asarray(a, dtype=np.float32))
          for a in (outputs1, outputs2, outputs3, outputs4)]
    s = np.ascontiguousarray(np.asarray(out_s, dtype=np.float32))
    tg = np.asarray(targets).astype(np.int64)

    idx = np.arange(B_FULL)
    g = np.stack([x[idx, tg] for x in xs], axis=1).astype(np.float64)  # [B,4]
    g_s = s[idx, tg].astype(np.float64)

    ident = np.eye(P, dtype=np.float32)
    negid = (-np.eye(P, dtype=np.float32)).astype(np.float32)
    in_maps = []
    for c in range(N_CORES):
        sl = slice(c * B_LOC, (c + 1) * B_LOC)
        m = {f"x{t}": xs[t][sl] for t in range(4)}
        m["s"] = s[sl]
        m["ident"] = ident
        m["negid"] = negid
        in_maps.append(m)

    results = _run_device(in_maps, trace=_trace)
    res_cores = [results.results[c]["res"] for c in range(N_CORES)]
    out = _host_combine(res_cores, g, g_s)
    if _return_results:
        return out, results
    return out

